# revision 1
# baseline (speedup 1.0000x reference)
"""Trainium2 Bass kernel for nn_CrossAttention (chunked local self-attn + full cross-attn).

Sharding: 8 cores = 2 batches x 4 query-row-blocks (512 rows each), fully SPMD,
no collectives.  Phase 1 (LN1 -> qkv -> chunked local attn (CHUNK=64) -> W_ao ->
+residual -> LN2 -> q_in) is query-row-independent.  Phase 2: each core
projects K/V from its batch's full x (4096 keys; 4x redundant within a batch)
and attends its 512 queries over all keys, streamed in 1024-key chunks.
Host only slices/transposes inputs and reassembles outputs.

Key implementation facts (hardware-validated):
- All heavy matmuls use float32r (full PE rate at N>=512, ~1e-4 rounding);
  plain f32 only for the tiny local-attention matmuls.
- HARD RULE: every matmul operand/output must sit at partition base 0.
  Mixing tile_position row offsets (0,0)/(64,0) across matmuls crashes the
  device (NRT unrecoverable), any dtype; f32r also rejects column offsets at
  compile time.  Hence all per-head operands live in separate base-0 tiles.
- Activations are kept feature-on-partitions ("transposed": lnT/qT/kT/qcT/
  kcT [feat, tok]) so projections use natural-layout weights as stationary
  operand; W_ao uses the transposed activation as *stationary* to emit the
  natural-layout output directly (no transpose-back).
- Softmax: exp on ScalarE with fused SCALE (max-subtraction skipped: logits
  are ~N(0, 0.2), safe); local attn normalizes via per-partition reciprocal
  + free-dim step-0 broadcast; cross attn gets denominators for free from a
  ones-column appended to V (row 64 of the [65, q] AV output) and broadcasts
  reciprocals across partitions with a ones-column matmul.
- Tile pools: PSUM is 8 banks, statically reserved per open pool (budget
  exactly); same-tag tiles that are alive simultaneously need distinct tags
  or bufs >= count (else scheduler deadlock).
"""

import os

import numpy as np

import concourse.bacc as bacc
import concourse.bass as bass
import concourse.mybir as mybir
import concourse.tile as tile
from concourse.bass_utils import run_bass_kernel_spmd
from concourse.masks import make_identity

F32 = mybir.dt.float32
F32R = mybir.dt.float32r
AF = mybir.ActivationFunctionType
ALU = mybir.AluOpType

H, DH, CHUNK = 8, 64, 64
DIM = 512
INNER = 512
EPS = 1e-5
SCALE = DH ** -0.5

T = 512          # query rows per core
NKT = 4096       # keys (full x length)
NF = DIM // 128  # feature tiles (4)
NT = T // 128    # token tiles per core (4)
KT_CHUNK = 1024  # cross-attn key-chunk
N_CHUNKS = NKT // KT_CHUNK


def _bcast_ap(dram_ap, parts):
    """[N] DRAM vector -> [parts, N] partition-broadcast AP (for DMA)."""
    return bass.AP(
        tensor=dram_ap.tensor,
        offset=dram_ap.offset,
        ap=[[0, parts]] + [list(x) for x in dram_ap.ap],
    )


def _layernorm(nc, pool, x_tiles, g_bc, b_bc, eps_tile, prefix, out_dtype=F32):
    """LayerNorm over free dim (512) of 4 [128,512] tiles. Returns new tiles."""
    out_tiles = []
    for tt in range(NT):
        x = x_tiles[tt]
        stats = pool.tile([128, 6], F32, name="ln_stats", tag="ln_stats")
        nc.vector.bn_stats(out=stats, in_=x)
        mv = pool.tile([128, 2], F32, name="ln_mv", tag="ln_mv")
        nc.vector.bn_aggr(out=mv, in_=stats)
        # rstd = 1/sqrt(var + eps)
        nc.scalar.activation(out=mv[:, 1:2], in_=mv[:, 1:2], func=AF.Sqrt,
                             bias=eps_tile, scale=1.0)
        nc.vector.reciprocal(out=mv[:, 1:2], in_=mv[:, 1:2])
        y = pool.tile([128, DIM], out_dtype, name=f"{prefix}{tt}", tag=f"{prefix}{tt}",
                      bufs=1)
        nc.vector.tensor_scalar(out=y, in0=x, scalar1=mv[:, 0:1], scalar2=mv[:, 1:2],
                                op0=ALU.subtract, op1=ALU.mult)
        nc.vector.tensor_tensor(out=y, in0=y, in1=g_bc, op=ALU.mult)
        nc.vector.tensor_tensor(out=y, in0=y, in1=b_bc, op=ALU.add)
        out_tiles.append(y)
    return out_tiles


def _transpose_to(nc, ps_pool, copy_engine, ident, src_tiles, dst_tiles):
    """dst[ft][:, tt*128:+128] = src[tt][:, ft*128:+128].T  (PE transposes)."""
    for tt in range(len(src_tiles)):
        for ft in range(NF):
            tp = ps_pool.tile([128, 128], F32, name="tposer", tag="tposer")
            nc.tensor.transpose(tp[:, :], src_tiles[tt][:, ft * 128:(ft + 1) * 128], ident)
            eng = nc.vector if copy_engine == "v" else nc.scalar
            if copy_engine == "v":
                nc.vector.tensor_copy(dst_tiles[ft][:, tt * 128:(tt + 1) * 128], tp)
            else:
                nc.scalar.activation(out=dst_tiles[ft][:, tt * 128:(tt + 1) * 128],
                                     in_=tp, func=AF.Copy)


def build_nc():
    nc = bacc.Bacc(None, target_bir_lowering=False)

    # ---------------- DRAM I/O ----------------
    qx_d = nc.dram_tensor("qx", [T, DIM], F32, kind="ExternalInput")
    xT_d = nc.dram_tensor("xT", [DIM, NKT], F32R, kind="ExternalInput")
    Wqkv_d = nc.dram_tensor("Wqkv", [DIM, 3 * INNER], F32R, kind="ExternalInput")
    Wao_d = nc.dram_tensor("Wao", [INNER, DIM], F32R, kind="ExternalInput")
    Wq_d = nc.dram_tensor("Wq", [DIM, INNER], F32R, kind="ExternalInput")
    Wkv_d = nc.dram_tensor("Wkv", [DIM, 2 * INNER], F32R, kind="ExternalInput")
    Wo_d = nc.dram_tensor("Wo", [INNER, DIM], F32R, kind="ExternalInput")
    ln1g_d = nc.dram_tensor("ln1g", [DIM], F32, kind="ExternalInput")
    ln1b_d = nc.dram_tensor("ln1b", [DIM], F32, kind="ExternalInput")
    ln2g_d = nc.dram_tensor("ln2g", [DIM], F32, kind="ExternalInput")
    ln2b_d = nc.dram_tensor("ln2b", [DIM], F32, kind="ExternalInput")
    bao_d = nc.dram_tensor("bao", [DIM], F32, kind="ExternalInput")
    bo_d = nc.dram_tensor("bo", [DIM], F32, kind="ExternalInput")
    qin_d = nc.dram_tensor("qin", [T, DIM], F32, kind="ExternalOutput")
    outT_d = nc.dram_tensor("outT", [DIM, T], F32, kind="ExternalOutput")

    with tile.TileContext(nc) as tc:
        with tc.tile_pool(name="singles", bufs=1) as singles, \
             tc.tile_pool(name="weights", bufs=1) as wpool, \
             tc.tile_pool(name="persist", bufs=1) as persist:

            ident = singles.tile([128, 128], F32)
            make_identity(nc, ident)
            eps_t = singles.tile([128, 1], F32)
            nc.vector.memset(eps_t, EPS)
            ones_t = singles.tile([1, 64], F32)
            nc.vector.memset(ones_t, 1.0)
            ones8 = singles.tile([128, 8, 1], F32)
            nc.vector.memset(ones8, 1.0)

            g1 = singles.tile([128, DIM], F32)
            b1 = singles.tile([128, DIM], F32)
            g2 = singles.tile([128, DIM], F32)
            b2 = singles.tile([128, DIM], F32)
            bao_bc = singles.tile([128, DIM], F32)
            nc.gpsimd.dma_start(out=g1, in_=_bcast_ap(ln1g_d[:], 128))
            nc.gpsimd.dma_start(out=b1, in_=_bcast_ap(ln1b_d[:], 128))
            nc.gpsimd.dma_start(out=g2, in_=_bcast_ap(ln2g_d[:], 128))
            nc.gpsimd.dma_start(out=b2, in_=_bcast_ap(ln2b_d[:], 128))
            nc.gpsimd.dma_start(out=bao_bc, in_=_bcast_ap(bao_d[:], 128))
            # bo as per-partition columns: bo[m*128 + p] -> bo_col[p, m]
            bo_col = singles.tile([128, NF], F32)
            nc.gpsimd.dma_start(out=bo_col, in_=bo_d[:].rearrange("(m p) -> p m", p=128))


            # qcT survives into the cross-attn phase (per-head, base-0)
            qcT = [persist.tile([64, T], F32R, name=f"qcTh{h}", tag=f"qcTh{h}") for h in range(H)]

            # =================== PHASE 1 ===================
            with tc.tile_pool(name="p1", bufs=1) as p1, \
                 tc.tile_pool(name="p1w", bufs=4) as p1w, \
                 tc.tile_pool(name="psT", bufs=2, space="PSUM") as psT, \
                 tc.tile_pool(name="psMM", bufs=2, space="PSUM") as psMM, \
                 tc.tile_pool(name="ps1", bufs=1, space="PSUM") as ps1:

                # ---- A. load qx FIRST (LN1's only data dependency), then weights
                qx_t = []
                for tt in range(NT):
                    x = p1.tile([128, DIM], F32, name=f"qx{tt}", tag=f"qx{tt}")
                    nc.sync.dma_start(out=x, in_=qx_d[tt * 128:(tt + 1) * 128, :])
                    qx_t.append(x)

                # phase-1 weights (released before cross-attn)
                Wqkv_sb = []
                Wao_sb = []
                Wq_sb = []
                for ft in range(NF):
                    w = p1.tile([128, 3 * INNER], F32R, name=f"wqkv{ft}", tag=f"wqkv{ft}")
                    nc.sync.dma_start(out=w, in_=Wqkv_d[ft * 128:(ft + 1) * 128, :])
                    Wqkv_sb.append(w)
                    w = p1.tile([128, DIM], F32R, name=f"wao{ft}", tag=f"wao{ft}")
                    nc.sync.dma_start(out=w, in_=Wao_d[ft * 128:(ft + 1) * 128, :])
                    Wao_sb.append(w)
                    w = p1.tile([128, INNER], F32R, name=f"wq{ft}", tag=f"wq{ft}")
                    nc.sync.dma_start(out=w, in_=Wq_d[ft * 128:(ft + 1) * 128, :])
                    Wq_sb.append(w)
                ln1 = _layernorm(nc, p1w, qx_t, g1, b1, eps_t, 'ln1_')

                # ---- B. transpose -> lnT (f32r)
                lnT = [p1.tile([128, T], F32R, name=f"lnT{ft}", tag=f"lnT{ft}") for ft in range(NF)]
                _transpose_to(nc, psT, "s", ident, ln1, lnT)

                # ---- C. qkv projections
                qT = [p1.tile([64, T], F32, name=f"qTh{h}", tag=f"qTh{h}") for h in range(H)]
                kT = [p1.tile([64, T], F32, name=f"kTh{h}", tag=f"kTh{h}") for h in range(H)]
                for m in range(8):  # 4 q tiles + 4 k tiles (transposed outputs)
                    ps = psMM.tile([128, T], F32, name="proj_ps", tag="proj_ps")
                    for ft in range(NF):
                        nc.tensor.matmul(ps[:, :],
                                         Wqkv_sb[ft][:, m * 128:(m + 1) * 128],
                                         lnT[ft][:, :],
                                         start=(ft == 0), stop=(ft == NF - 1))
                    dst = qT if m < 4 else kT
                    mm = m % 4
                    nc.scalar.activation(out=dst[2 * mm], in_=ps[0:64, :], func=AF.Copy)
                    nc.scalar.activation(out=dst[2 * mm + 1], in_=ps[64:128, :],
                                         func=AF.Copy)
                # v in natural layout, split per 64-row chunk (partition base 0)
                v_loc = [p1.tile([64, INNER], F32, name=f"vloc{c}", tag=f"vloc{c}") for c in range(T // CHUNK)]
                for tt in range(NT):
                    ps = psMM.tile([128, INNER], F32, name="proj_ps", tag="proj_ps")
                    for ft in range(NF):
                        nc.tensor.matmul(ps[:, :],
                                         lnT[ft][:, tt * 128:(tt + 1) * 128],
                                         Wqkv_sb[ft][:, 2 * INNER:3 * INNER],
                                         start=(ft == 0), stop=(ft == NF - 1))
                    nc.scalar.activation(out=v_loc[2 * tt], in_=ps[0:64, :],
                                         func=AF.Copy)
                    nc.scalar.activation(out=v_loc[2 * tt + 1], in_=ps[64:128, :],
                                         func=AF.Copy)

                # ---- D. chunked local attention -> oT_local (transposed, f32r)
                oT_local = [p1.tile([128, T], F32R, name=f"oTl{m}", tag=f"oTl{m}") for m in range(NF)]
                if os.environ.get("KSKIP_LOCAL", "0") == "1":
                    for m in range(NF):
                        nc.vector.tensor_copy(oT_local[m], qT[m])
                for cp in range(NT) if os.environ.get("KSKIP_LOCAL", "0") != "1" else []:
                    # two 64-row score tiles (chunk A / chunk B), all outputs base-0
                    s_ps = [ps1.tile([64, H, CHUNK], F32, name=f"s_loc{c01}",
                                     tag=f"s_loc{c01}") for c01 in range(2)]
                    for h in range(H):
                        for c01 in range(2):
                            qs = qT[h][:, cp * 128 + c01 * 64: cp * 128 + (c01 + 1) * 64]
                            ks = kT[h][:, cp * 128 + c01 * 64: cp * 128 + (c01 + 1) * 64]
                            nc.tensor.matmul(s_ps[c01][:, h, :], qs, ks,
                                             start=True, stop=True,
                                             tile_position=(0, 0))
                    a_sb = [None, None]
                    for c01 in range(2):
                        a = p1w.tile([64, H, CHUNK], F32, name="a_loc", tag="a_loc")
                        nc.scalar.activation(out=a, in_=s_ps[c01], func=AF.Exp,
                                             scale=SCALE)
                        sums = p1w.tile([64, H], F32, name="sums_loc", tag="sums_loc")
                        nc.vector.tensor_reduce(out=sums, in_=a,
                                                axis=mybir.AxisListType.X, op=ALU.add)
                        nc.vector.reciprocal(out=sums, in_=sums)
                        nc.vector.tensor_tensor(out=a, in0=a,
                                                in1=sums.broadcast_to((64, H, CHUNK)),
                                                op=ALU.mult)
                        a_sb[c01] = a
                    for h in range(H):
                        hp, hr = h // 2, (h % 2) * 64
                        av_ps = ps1.tile([64, 128], F32, name="av_loc", tag="av_loc", bufs=2)
                        for c01 in range(2):
                            aT_ps = psT.tile([64, 64], F32, name="tposer", tag="tposer")
                            nc.tensor.transpose(aT_ps[:, :], a_sb[c01][:, h, :], ident[0:64, 0:64])
                            aT = p1w.tile([64, 64], F32, name="aT_sb", tag="aT_sb")
                            nc.scalar.activation(out=aT, in_=aT_ps, func=AF.Copy)
                            vs = v_loc[cp * 2 + c01][:, h * 64:(h + 1) * 64]
                            nc.tensor.matmul(av_ps[:, c01 * 64:(c01 + 1) * 64],
                                             vs, aT, start=True, stop=True,
                                             tile_position=(0, 0))
                        nc.vector.tensor_copy(
                            oT_local[hp][hr:hr + 64, cp * 128:(cp + 1) * 128], av_ps)

                # ---- E. W_ao projection: natural output directly
                # (oT_local tile as stationary operand) + residual + bias
                ao = [p1.tile([128, DIM], F32, name=f"ao{tt}", tag=f"ao{tt}") for tt in range(NT)]
                for tt in range(NT):
                    ps = psMM.tile([128, T], F32, name="proj_ps", tag="proj_ps")
                    for ft in range(NF):
                        nc.tensor.matmul(ps[:, :],
                                         oT_local[ft][:, tt * 128:(tt + 1) * 128],
                                         Wao_sb[ft][:, :],
                                         start=(ft == 0), stop=(ft == NF - 1))
                    nc.vector.tensor_tensor(out=ao[tt], in0=ps, in1=bao_bc, op=ALU.add)
                    nc.vector.tensor_tensor(out=ao[tt], in0=ao[tt], in1=qx_t[tt], op=ALU.add)

                # ---- F. LN2 -> q_in (output) ; G. transpose -> qinT
                qin = _layernorm(nc, p1w, ao, g2, b2, eps_t, 'qin_')
                for tt in range(NT):
                    nc.sync.dma_start(out=qin_d[tt * 128:(tt + 1) * 128, :], in_=qin[tt])
                qinT = [p1.tile([128, T], F32R, name=f"qinT{ft}", tag=f"qinT{ft}") for ft in range(NF)]
                _transpose_to(nc, psT, "s", ident, qin, qinT)

                # ---- H. W_q projection -> qcT (persists)
                for m in range(NF):
                    ps = psMM.tile([128, T], F32, name="proj_ps", tag="proj_ps")
                    for ft in range(NF):
                        nc.tensor.matmul(ps[:, :],
                                         Wq_sb[ft][:, m * 128:(m + 1) * 128],
                                         qinT[ft][:, :],
                                         start=(ft == 0), stop=(ft == NF - 1))
                    nc.vector.tensor_copy(qcT[2 * m], ps[0:64, :])
                    nc.vector.tensor_copy(qcT[2 * m + 1], ps[64:128, :])

            # =================== PHASE 2: cross-attention ===================
            _PHASE = os.environ.get("KPHASE", "all")
            if _PHASE == "p1":
                with tc.tile_pool(name="dummy", bufs=1) as dummy:
                    for m in range(NF):
                        z = dummy.tile([64, T], F32, name=f"z{m}", tag=f"z{m}")
                        nc.vector.tensor_copy(z, qcT[m].bitcast(F32))
                        nc.sync.dma_start(out=outT_d[m * 128:m * 128 + 64, :], in_=z)
            NKTT = KT_CHUNK // 128  # kt tiles per chunk
            if _PHASE != "p1":
              with tc.tile_pool(name="wx", bufs=1) as wxpool:
                Wkv_sb = []
                Wo_sb = []
                for ft in range(NF):
                    w = wxpool.tile([128, 2 * INNER], F32R, name=f"wkv{ft}", tag=f"wkv{ft}")
                    nc.sync.dma_start(out=w, in_=Wkv_d[ft * 128:(ft + 1) * 128, :])
                    Wkv_sb.append(w)
                    w = wxpool.tile([128, DIM], F32R, name=f"wo{ft}", tag=f"wo{ft}")
                    nc.sync.dma_start(out=w, in_=Wo_d[ft * 128:(ft + 1) * 128, :])
                    Wo_sb.append(w)
                oT_sb = [wxpool.tile([65, T], F32, name=f"oT{h}", tag=f"oT{h}")
                         for h in range(H)]
                with tc.tile_pool(name="xc", bufs=2) as xc_pool, \
                     tc.tile_pool(name="kc", bufs=2) as kc_pool, \
                     tc.tile_pool(name="vc", bufs=12) as vc_pool, \
                     tc.tile_pool(name="pa", bufs=4) as pa_pool, \
                     tc.tile_pool(name="ps_s", bufs=2, space="PSUM") as ps_s, \
                     tc.tile_pool(name="ps_o", bufs=2, space="PSUM") as ps_o, \
                     tc.tile_pool(name="ps_p", bufs=2, space="PSUM") as ps_p:
                    for chunk in range(N_CHUNKS):
                        k0 = chunk * KT_CHUNK
                        xTc = []
                        for ft in range(NF):
                            xt = xc_pool.tile([128, KT_CHUNK], F32R, name=f"xTc{ft}", tag=f"xTc{ft}")
                            nc.sync.dma_start(
                                out=xt, in_=xT_d[ft * 128:(ft + 1) * 128, k0:k0 + KT_CHUNK])
                            xTc.append(xt)
                        # K^T projection: kcT[m] [128 (2 heads), kt]
                        kcT = [kc_pool.tile([64, KT_CHUNK], F32R, name=f"kcTh{h}",
                                            tag=f"kcTh{h}") for h in range(H)]
                        for m in range(NF):
                            for nchunk in range(KT_CHUNK // 512):
                                kc_ps = ps_p.tile([128, 512], F32, name="proj_ps", tag="proj_ps")
                                for ft in range(NF):
                                    nc.tensor.matmul(
                                        kc_ps[:, :],
                                        Wkv_sb[ft][:, m * 128:(m + 1) * 128],
                                        xTc[ft][:, nchunk * 512:(nchunk + 1) * 512],
                                        start=(ft == 0), stop=(ft == NF - 1))
                                sl = slice(nchunk * 512, (nchunk + 1) * 512)
                                nc.vector.tensor_copy(kcT[2 * m][:, sl], kc_ps[0:64, :])
                                nc.vector.tensor_copy(kcT[2 * m + 1][:, sl], kc_ps[64:128, :])
                        # V projection (natural) + ones column -> v_aug [128, H, 65]
                        v_aug = []
                        for kt in range(NKTT):
                            vp = ps_p.tile([128, INNER], F32, name="proj_ps", tag="proj_ps")
                            for ft in range(NF):
                                nc.tensor.matmul(
                                    vp[:, :],
                                    xTc[ft][:, kt * 128:(kt + 1) * 128],
                                    Wkv_sb[ft][:, INNER:2 * INNER],
                                    start=(ft == 0), stop=(ft == NF - 1))
                            va = vc_pool.tile([128, H, 65], F32R, name="v_aug", tag="v_aug")
                            nc.vector.tensor_copy(
                                va[:, :, 0:64],
                                vp[:, :].rearrange("p (h d) -> p h d", h=H))
                            nc.vector.tensor_copy(va[:, :, 64:65], ones8)
                            v_aug.append(va)
                        # attention per head
                        GSZ = 2
                        NG = NKTT // GSZ
                        for h in range(H):
                            o_ps = ps_o.tile([65, T], F32, name="o_ps", tag="o_ps")
                            for g in range(NG):  # groups of GSZ kt-tiles
                                s_ps = ps_s.tile([128, GSZ, T], F32, name="s_ps",
                                                 tag="s_ps")
                                for j in range(GSZ):
                                    kt = g * GSZ + j
                                    nc.tensor.matmul(
                                        s_ps[:, j, :],
                                        kcT[h][:, kt * 128:(kt + 1) * 128],
                                        qcT[h][:, :],
                                        start=True, stop=True,
                                        tile_position=(0, 0))
                                a_sb = pa_pool.tile([128, GSZ, T], F32R, name="a_sb",
                                                    tag="a_sb")
                                nc.scalar.activation(out=a_sb, in_=s_ps, func=AF.Exp,
                                                     scale=SCALE)
                                for j in range(GSZ):
                                    kt = g * GSZ + j
                                    nc.tensor.matmul(
                                        o_ps[:, :],
                                        v_aug[kt][:, h, :],
                                        a_sb[:, j, :],
                                        start=(g == 0 and j == 0),
                                        stop=(g == NG - 1 and j == GSZ - 1))
                            if chunk == 0:
                                nc.vector.tensor_copy(oT_sb[h], o_ps)
                            else:
                                nc.vector.tensor_tensor(out=oT_sb[h], in0=oT_sb[h],
                                                        in1=o_ps, op=ALU.add)

                # =================== normalize + W_o ===================
                with tc.tile_pool(name="fin", bufs=1) as fin, \
                     tc.tile_pool(name="ps_f", bufs=2, space="PSUM") as ps_f:
                    oT_norm = [fin.tile([128, T], F32R, name=f"oTn{m}", tag=f"oTn{m}") for m in range(NF)]
                    for h in range(H):
                        hp, hr = h // 2, (h % 2) * 64
                        rec = fin.tile([1, T], F32, name="rec", tag="rec", bufs=2)
                        nc.vector.reciprocal(rec, oT_sb[h][64:65, :])
                        bc_ps = ps_f.tile([64, T], F32, name="bc_ps", tag="bc_ps")
                        nc.tensor.matmul(bc_ps[:, :], ones_t[0:1, :], rec[0:1, :],
                                         start=True, stop=True)
                        nc.vector.tensor_tensor(out=oT_norm[hp][hr:hr + 64, :],
                                                in0=oT_sb[h][0:64, :], in1=bc_ps,
                                                op=ALU.mult)
                    for m in range(NF):
                        ps = ps_f.tile([128, T], F32, name="out_ps", tag="out_ps")
                        for ft in range(NF):
                            nc.tensor.matmul(ps[:, :],
                                             Wo_sb[ft][:, m * 128:(m + 1) * 128],
                                             oT_norm[ft][:, :],
                                             start=(ft == 0), stop=(ft == NF - 1))
                        ot = fin.tile([128, T], F32, name="outT_sb", tag="outT_sb", bufs=2)
                        nc.vector.tensor_scalar(out=ot, in0=ps, scalar1=bo_col[:, m:m + 1],
                                                scalar2=None, op0=ALU.add)
                        nc.sync.dma_start(out=outT_d[m * 128:(m + 1) * 128, :], in_=ot)

    nc.finalize()
    return nc


_NC_CACHE = {}


def kernel(x, q_x, ln1_g, ln1_b, W_qkv, W_ao, b_ao, ln2_g, ln2_b,
           W_q, W_kv, W_o, b_o):
    B, NQ, _ = q_x.shape
    n_blocks = 4
    rows = NQ // n_blocks

    if "nc" not in _NC_CACHE:
        _NC_CACHE["nc"] = build_nc()
    nc = _NC_CACHE["nc"]

    xT = np.ascontiguousarray(x.transpose(0, 2, 1)).astype(np.float32)
    common = {
        "Wqkv": np.ascontiguousarray(W_qkv, np.float32),
        "Wao": np.ascontiguousarray(W_ao, np.float32),
        "Wq": np.ascontiguousarray(W_q, np.float32),
        "Wkv": np.ascontiguousarray(W_kv, np.float32),
        "Wo": np.ascontiguousarray(W_o, np.float32),
        "ln1g": np.ascontiguousarray(ln1_g, np.float32),
        "ln1b": np.ascontiguousarray(ln1_b, np.float32),
        "ln2g": np.ascontiguousarray(ln2_g, np.float32),
        "ln2b": np.ascontiguousarray(ln2_b, np.float32),
        "bao": np.ascontiguousarray(b_ao, np.float32),
        "bo": np.ascontiguousarray(b_o, np.float32),
    }
    in_maps = []
    for c in range(8):
        b, r = c // n_blocks, c % n_blocks
        m = dict(common)
        m["qx"] = np.ascontiguousarray(q_x[b, r * rows:(r + 1) * rows, :], np.float32)
        m["xT"] = xT[b]
        in_maps.append(m)

    res = run_bass_kernel_spmd(nc, in_maps, core_ids=list(range(8)))

    out = np.empty((B, NQ, DIM), np.float32)
    q_in = np.empty((B, NQ, DIM), np.float32)
    for c in range(8):
        b, r = c // n_blocks, c % n_blocks
        q_in[b, r * rows:(r + 1) * rows, :] = res.results[c]["qin"]
        out[b, r * rows:(r + 1) * rows, :] = res.results[c]["outT"].T
    return (out, q_in)

